# revision 8
# baseline (speedup 1.0000x reference)
"""AdderNet BasicBlock (Adder2D 3x3 + BatchNorm(train) + ReLU) on 8 TRN2 cores.

Problem: x[4,64,32,32], weight[64,64,3,3], gamma[64], beta[64] ->
    out[b,o,y,x] = relu(BN_train(-sum_{c,ky,kx} |x_pad[b,c,y+ky,x+kx] - w[o,c,ky,kx]|))

Sharding: output channels O=64 split 8 per core. BatchNorm stats are per-channel
over (B,H,W), so each core's 8 channels are fully self-contained: no collectives.

Per-core dataflow (all shapes hardcoded):
  - Host ships x as TWO pre-padded f16 images [128, 2*34*34]: partition
    p=(h,c) with h=b%2, free (u=b//2, y, x) zero-padded; the second image is
    shifted by one f16 element so odd-offset taps keep 4-byte-aligned APs for
    the DVE 4x mode. No on-chip f32->f16 copies or border memsets.
  - For each (o, tap): D[128, 2, 32, 32] f16 = |view - w[o, c, tap]|:
    ~29 taps on ACT (activation Abs with per-partition bias=-w, one op,
    ~1.9us each - ACT has no fast mode), the rest on DVE (tensor_scalar
    subtract at 4x ~594ns, then sign-bit clear via bitwise_and 0x7FFF on the
    u16 view, also 4x). A few tap pairs are pre-summed on DVE (tensor_tensor
    add f16, 2x ~1.1us) so the PE streams fewer columns; engine loads are
    balanced ACT ~= DVE ~= PE.
  - PE reduces over partitions, accumulating into one persistent PSUM tile
    S[32, 1024] (row = o_local*4 + b). lhsT is a one-hot f16 selector so every
    matmul writes base partition 0 (hw constraint); f16 matmul = 512 cols
    ~212ns.
  - Epilogue (all DVE; no ACT table loads): row sums via tensor_reduce, row
    sum-of-squares via scalar_tensor_tensor mult+accum_out, one tiny matmul
    with a selector to fold the 4 batch rows per channel, var = E[S^2]-E[S]^2,
    rstd via AluOp pow(-0.5), then out = relu(A*S + b') with A = -gamma*rstd
    and b' = beta - A*mean folded into a dual-op tensor_scalar (also drains
    PSUM), relu via tensor_scalar max 0.

kernel() is self-contained: builds the Bass program once, shards inputs on host,
runs via bass_utils.run_bass_kernel_spmd on cores 0..7, reassembles full output.
"""

import functools
import os

import numpy as np

B, C, O, H, W = 4, 64, 64, 32, 32
K, PAD = 3, 1
HP, WP = H + 2 * PAD, W + 2 * PAD  # 34, 34
L = H * W  # 1024
SPP = HP * WP  # 1156 padded spatial per batch
NCORES = 8
O_PER = O // NCORES  # 8
NB2 = B // 2  # bpairs
EPS = 1e-5
NSTAT = O_PER * B  # 32 rows of S
NPIX = B * L  # 4096 values per channel for BN stats

# engine split knobs: 72 taps per core (8 o x 9 taps).
N_ACT_OPS = int(os.environ.get("KRN_ACT_OPS", "29"))  # taps on ACT (1 op, ~1.9us)
N_MERGE = int(os.environ.get("KRN_MERGE", "5"))  # DVE tap-pairs pre-summed
DUAL_OP = int(os.environ.get("KRN_DUAL", "0"))  # sub+and in one tensor_scalar
D_BUFS = int(os.environ.get("KRN_D_BUFS", "10"))


def _engine_schedule(n_ops: int, n_act: int):
    """Return list of 'v'/'a' of length n_ops, interleaving engines evenly."""
    n_a = min(n_act, n_ops)
    n_v = n_ops - n_a
    counts = {"v": n_v, "a": n_a}
    acc = {k: 0.0 for k in counts}
    sched = []
    for _ in range(n_ops):
        for k in counts:
            acc[k] += counts[k] / n_ops
        pick = max(acc, key=lambda k: acc[k])
        acc[pick] -= 1.0
        sched.append(pick)
    return sched


def _merge_plan(sched):
    """Distribute N_MERGE pre-sum pairs round-robin over o's with >=2 DVE taps."""
    pairs = {o: [] for o in range(O_PER)}
    remaining = N_MERGE
    rnd = 0
    while remaining > 0 and rnd < 4:
        for o in range(O_PER):
            if remaining <= 0:
                break
            dve = [t for t in range(9) if sched[o * 9 + t] == "v"]
            used = {t for p in pairs[o] for t in p}
            free = [t for t in dve if t not in used]
            if len(free) >= 2:
                pairs[o].append((free[0], free[1]))
                remaining -= 1
        rnd += 1
    return pairs


def _emit_main(nc, tc, mybir, xph4, xpho4, wcols, nwcols, mselh, dpool, ps, sched, pairs):
    from concourse import mybir as _mb

    f16 = _mb.dt.float16
    u16 = _mb.dt.uint16

    def absdiff(o, tap):
        """Emit |x - w| for (o, tap) -> f16 tile d[128, NB2, H, W]."""
        ky, kx = tap // 3, tap % 3
        idx = o * 9 + tap
        eng = sched[idx]
        if kx == 1:
            # odd flat offset: read the +1-shifted image at kx-1 (even, aligned)
            view = xpho4[:, :, ky : ky + H, kx - 1 : kx - 1 + W]
        else:
            view = xph4[:, :, ky : ky + H, kx : kx + W]
        if eng == "a":
            # ACT reads either image; alignment irrelevant for ACT
            aview = xph4[:, :, ky : ky + H, kx : kx + W]
            d = dpool.tile([128, NB2, H, W], f16, tag="da", name=f"da{idx}")
            nc.scalar.activation(
                out=d[:], in_=aview, func=_mb.ActivationFunctionType.Abs,
                bias=nwcols[:, idx : idx + 1], scale=1.0,
            )
            return d
        if DUAL_OP:
            d = dpool.tile([128, NB2, H, W], f16, tag="dv", name=f"dv{idx}")
            nc.vector.tensor_scalar(
                out=d[:], in0=view, scalar1=wcols[:, idx : idx + 1],
                scalar2=0x7FFF, op0=_mb.AluOpType.subtract,
                op1=_mb.AluOpType.bitwise_and,
            )
            return d
        d1 = dpool.tile([128, NB2, H, W], f16, tag="d1", name=f"d1_{idx}")
        nc.vector.tensor_scalar_sub(d1[:], view, wcols[:, idx : idx + 1])
        d = dpool.tile([128, NB2, H, W], f16, tag="dv", name=f"dv{idx}")
        nc.vector.tensor_scalar(
            out=d[:].bitcast(u16), in0=d1[:].bitcast(u16),
            scalar1=0x7FFF, scalar2=None, op0=_mb.AluOpType.bitwise_and,
        )
        return d

    first = [True, True]

    def mm(o, d, last):
        d2 = d.rearrange("p u a b -> p (u a b)")
        for u in range(NB2):
            for half in range(2):
                nc.tensor.matmul(
                    ps[half][:, :],
                    lhsT=mselh[:, (o * 2 + u) * NSTAT : (o * 2 + u + 1) * NSTAT],
                    rhs=d2[:, u * L + half * 512 : u * L + half * 512 + 512],
                    start=first[half],
                    stop=(last and u == NB2 - 1),
                )
                first[half] = False

    for o in range(O_PER):
        last_of_o = o == O_PER - 1
        merged = {t for p in pairs[o] for t in p}
        # plain taps first: keeps the in-order PE queue fed while the
        # DVE merge chains (abs, abs, add) complete in the background
        plain = [absdiff(o, tap) for tap in range(9) if tap not in merged]
        for i, d in enumerate(plain):
            mm(o, d, last_of_o and not pairs[o] and i == len(plain) - 1)
        for k, (t1, t2) in enumerate(pairs[o]):
            di = absdiff(o, t1)
            dj = absdiff(o, t2)
            dsum = dpool.tile([128, NB2, H, W], f16, tag="dsum",
                              name=f"dsum{o}_{t1}")
            nc.vector.tensor_add(dsum[:], di[:], dj[:])
            mm(o, dsum, last_of_o and k == len(pairs[o]) - 1)


@functools.lru_cache(maxsize=4)
def _build_program(bench_iters=0):
    from contextlib import ExitStack

    import concourse.tile as tile
    from concourse import bacc, mybir

    f32 = mybir.dt.float32
    f16 = mybir.dt.float16

    nc = bacc.Bacc("TRN2", target_bir_lowering=False, debug=False)

    # pre-padded f16 images: partition p=(h,c), free=(u, y, x); xpho shifted +1
    xph_t = nc.dram_tensor("xph", (128, NB2 * SPP), f16, kind="ExternalInput")
    xpho_t = nc.dram_tensor("xpho", (128, NB2 * SPP), f16, kind="ExternalInput")
    # wpack[:, :72] = wcols (w[o_g, p%64, tap]), [:, 72:144] = -wcols
    wpack_t = nc.dram_tensor("wpack", (128, 2 * O_PER * 9), f32, kind="ExternalInput")
    # mselh[p, (o*2+u)*32 + j] = 1.0 iff j == o*4 + 2u + p//64
    mselh_t = nc.dram_tensor("mselh", (128, O_PER * 2 * NSTAT), f16, kind="ExternalInput")
    # spack[:, :32] = osel, [:, 32] = -gamma col, [:, 33] = beta col
    spack_t = nc.dram_tensor("spack", (NSTAT, NSTAT + 2), f32, kind="ExternalInput")
    out_t = nc.dram_tensor("out", (NSTAT, L), f32, kind="ExternalOutput")

    sched = _engine_schedule(O_PER * 9, N_ACT_OPS)
    pairs = _merge_plan(sched)

    with tile.TileContext(nc) as tc, ExitStack() as ctx:
        consts = ctx.enter_context(tc.tile_pool(name="consts", bufs=1))
        dpool = ctx.enter_context(tc.tile_pool(name="dpool", bufs=D_BUFS))
        spool = ctx.enter_context(tc.tile_pool(name="spool", bufs=2))
        psum_main = ctx.enter_context(tc.tile_pool(name="psum_main", bufs=1, space="PSUM"))
        psum_stat = ctx.enter_context(tc.tile_pool(name="psum_stat", bufs=2, space="PSUM"))

        # ---- inputs to SBUF ----
        xph = consts.tile([128, NB2 * SPP], f16)
        xpho = consts.tile([128, NB2 * SPP], f16)
        wpack = consts.tile([128, 2 * O_PER * 9], f32)
        mselh = consts.tile([128, O_PER * 2 * NSTAT], f16)
        spack = consts.tile([NSTAT, NSTAT + 2], f32)
        nc.sync.dma_start(out=xph[:], in_=xph_t[:, :])
        nc.sync.dma_start(out=xpho[:], in_=xpho_t[:, :])
        nc.sync.dma_start(out=wpack[:], in_=wpack_t[:, :])
        nc.sync.dma_start(out=mselh[:], in_=mselh_t[:, :])
        nc.sync.dma_start(out=spack[:], in_=spack_t[:, :])
        wcols = wpack[:, 0 : O_PER * 9]
        nwcols = wpack[:, O_PER * 9 : 2 * O_PER * 9]
        osel = spack[:, 0:NSTAT]
        gcol = spack[:, NSTAT : NSTAT + 1]
        bcol = spack[:, NSTAT + 1 : NSTAT + 2]

        xph4 = xph.rearrange("p (u a b) -> p u a b", u=NB2, a=HP, b=WP)
        xpho4 = xpho.rearrange("p (u a b) -> p u a b", u=NB2, a=HP, b=WP)

        # ---- main loop: S[o*4+b, l] accumulates over taps in two PSUM halves
        ps_big = psum_main.tile([NSTAT, 2 * 512], f32, name="ps_big")
        ps = [ps_big[:, h * 512 : h * 512 + 512] for h in range(2)]
        import contextlib

        loop_cm = (
            tc.For_i(0, bench_iters, 1) if bench_iters else contextlib.nullcontext()
        )
        with loop_cm:
            _emit_main(nc, tc, mybir, xph4, xpho4, wcols, nwcols, mselh,
                       dpool, ps, sched, pairs)

        # ---- epilogue: BN stats + normalize + relu, all on DVE ----
        sums2 = spool.tile([NSTAT, 2], f32, tag="sums2")
        # drain S from PSUM to SBUF; accum_out gives the row sums for free
        scp = spool.tile([NSTAT, L], f32, tag="scp")
        nc.vector.tensor_scalar(
            out=scp[:], in0=ps_big[:], scalar1=1.0, scalar2=None,
            op0=mybir.AluOpType.mult, op1=mybir.AluOpType.add,
            accum_out=sums2[:, 0:1],
        )
        scr = spool.tile([NSTAT, L], f32, tag="scr")
        nc.vector.scalar_tensor_tensor(
            out=scr[:], in0=scp[:], scalar=1.0, in1=scp[:],
            op0=mybir.AluOpType.mult, op1=mybir.AluOpType.mult,
            accum_out=sums2[:, 1:2],
        )
        stat_ps = psum_stat.tile([NSTAT, 2], f32, tag="statps")
        nc.tensor.matmul(stat_ps[:], lhsT=osel, rhs=sums2[:], start=True, stop=True)
        mom = spool.tile([NSTAT, 2], f32, tag="mom")
        nc.vector.tensor_scalar_mul(mom[:], stat_ps[:], 1.0 / NPIX)  # [mean, E[S^2]]
        mean = mom[:, 0:1]
        nvar = spool.tile([NSTAT, 1], f32, tag="nvar")
        # (mean*mean) - E[S^2] = -var
        nc.vector.scalar_tensor_tensor(
            out=nvar[:], in0=mean, scalar=mean, in1=mom[:, 1:2],
            op0=mybir.AluOpType.mult, op1=mybir.AluOpType.subtract,
        )
        epsc = spool.tile([NSTAT, 1], f32, tag="epsc")
        nc.vector.memset(epsc[:], EPS)
        rstd = spool.tile([NSTAT, 1], f32, tag="rstd")
        # rstd = 1/sqrt(|-nvar + eps|) = 1/sqrt(var + eps)
        nc.scalar.activation(
            out=rstd[:], in_=nvar[:],
            func=mybir.ActivationFunctionType.Abs_reciprocal_sqrt,
            bias=epsc[:], scale=-1.0,
        )
        acol = spool.tile([NSTAT, 1], f32, tag="acol")
        nc.vector.tensor_mul(acol[:], gcol, rstd[:])  # A = -gamma*rstd
        qcol = spool.tile([NSTAT, 1], f32, tag="qcol")
        # q = A*mean - beta ; out = relu(A*S - q)
        nc.vector.scalar_tensor_tensor(
            out=qcol[:], in0=acol[:], scalar=mean, in1=bcol,
            op0=mybir.AluOpType.mult, op1=mybir.AluOpType.subtract,
        )
        t1 = spool.tile([NSTAT, L], f32, tag="t1")
        nc.vector.tensor_scalar(
            out=t1[:], in0=scp[:], scalar1=acol[:], scalar2=qcol[:],
            op0=mybir.AluOpType.mult, op1=mybir.AluOpType.subtract,
        )
        outf = spool.tile([NSTAT, L], f32, tag="outf")
        nc.vector.tensor_scalar_max(outf[:], t1[:], 0.0)
        nc.sync.dma_start(out=out_t[:, :], in_=outf[:])

    nc.compile()
    return nc


def _host_inputs(x, weight, gamma, beta):
    """Build the 8 per-core input maps."""
    x = np.ascontiguousarray(x, dtype=np.float32)
    weight = np.asarray(weight, dtype=np.float32)
    gamma = np.asarray(gamma, dtype=np.float32)
    beta = np.asarray(beta, dtype=np.float32)

    # padded f16 images, shared by all cores
    img = np.zeros((2, C, NB2, HP, WP), dtype=np.float16)
    for b in range(B):
        h, u = b % 2, b // 2
        img[h, :, u, PAD : PAD + H, PAD : PAD + W] = x[b].astype(np.float16)
    xph = img.reshape(128, NB2 * SPP)
    xpho = np.zeros_like(xph)
    xpho[:, : NB2 * SPP - 1] = xph[:, 1:]

    msel = np.zeros((128, O_PER * 2 * NSTAT), dtype=np.float32)
    for o in range(O_PER):
        for u in range(NB2):
            for p_half in range(2):
                j = o * 4 + 2 * u + p_half
                col = (o * 2 + u) * NSTAT + j
                msel[p_half * 64 : (p_half + 1) * 64, col] = 1.0
    osel = np.zeros((NSTAT, NSTAT), dtype=np.float32)
    for p in range(NSTAT):
        for m in range(NSTAT):
            if p // B == m // B:
                osel[p, m] = 1.0

    in_maps = []
    for core in range(NCORES):
        osl = slice(core * O_PER, (core + 1) * O_PER)
        w = weight[osl]  # [8, 64, 3, 3]
        # wcols[p, o*9+tap] = w[o, p%64, tap//3, tap%3]
        wc = w.reshape(O_PER, C, 9).transpose(1, 0, 2).reshape(C, O_PER * 9)
        wcols = np.concatenate([wc, wc], axis=0).astype(np.float32)  # [128, 72]
        wpack = np.concatenate([wcols, -wcols], axis=1)  # [128, 144]
        gcol = np.repeat(-gamma[osl], B).reshape(NSTAT, 1).astype(np.float32)
        bcol = np.repeat(beta[osl], B).reshape(NSTAT, 1).astype(np.float32)
        spack = np.concatenate([osel, gcol, bcol], axis=1)  # [32, 34]
        in_maps.append(
            {
                "xph": xph,
                "xpho": xpho,
                "wpack": np.ascontiguousarray(wpack),
                "mselh": msel.astype(np.float16),
                "spack": np.ascontiguousarray(spack),
            }
        )
    return in_maps


def _assemble(results):
    out = np.empty((B, O, H, W), dtype=np.float32)
    for core, res in enumerate(results):
        arr = res["out"].reshape(O_PER, B, H, W)  # row = o*4+b
        out[:, core * O_PER : (core + 1) * O_PER] = arr.transpose(1, 0, 2, 3)
    return out


def kernel(x, weight, gamma, beta, _trace=False):
    from concourse import bass_utils

    nc = _build_program()
    in_maps = _host_inputs(x, weight, gamma, beta)
    res = bass_utils.run_bass_kernel_spmd(
        nc, in_maps, core_ids=list(range(NCORES)), trace=_trace
    )
    out = _assemble(res.results)
    if _trace:
        return out, res
    return out


# revision 9
# speedup vs baseline: 1.1360x; 1.1360x over previous
"""AdderNet BasicBlock (Adder2D 3x3 + BatchNorm(train) + ReLU) on 8 TRN2 cores.

Problem: x[4,64,32,32], weight[64,64,3,3], gamma[64], beta[64] ->
    out[b,o,y,x] = relu(BN_train(-sum_{c,ky,kx} |x_pad[b,c,y+ky,x+kx] - w[o,c,ky,kx]|))

Sharding: output channels O=64 split 8 per core. BatchNorm stats are per-channel
over (B,H,W), so each core's 8 channels are fully self-contained: no collectives.

Per-core dataflow (all shapes hardcoded):
  - Host ships x as TWO pre-padded f16 images [128, 2*34*34]: partition
    p=(h,c) with h=b%2, free (u=b//2, y, x) zero-padded; the second image is
    shifted by one f16 element so odd-offset taps keep 4-byte-aligned APs for
    the DVE 4x mode. No on-chip f32->f16 copies or border memsets.
  - For each (o, tap): D[128, 2, 32, 32] f16 = |view - w[o, c, tap]|:
    ~29 taps on ACT (activation Abs with per-partition bias=-w, one op,
    ~1.9us each - ACT has no fast mode), the rest on DVE (tensor_scalar
    subtract at 4x ~594ns, then sign-bit clear via bitwise_and 0x7FFF on the
    u16 view, also 4x). A few tap pairs are pre-summed on DVE (tensor_tensor
    add f16, 2x ~1.1us) so the PE streams fewer columns; engine loads are
    balanced ACT ~= DVE ~= PE.
  - PE reduces over partitions, accumulating into one persistent PSUM tile
    S[32, 1024] (row = o_local*4 + b). lhsT is a one-hot f16 selector so every
    matmul writes base partition 0 (hw constraint); f16 matmul = 512 cols
    ~212ns.
  - Epilogue (all DVE; no ACT table loads): row sums via tensor_reduce, row
    sum-of-squares via scalar_tensor_tensor mult+accum_out, one tiny matmul
    with a selector to fold the 4 batch rows per channel, var = E[S^2]-E[S]^2,
    rstd via AluOp pow(-0.5), then out = relu(A*S + b') with A = -gamma*rstd
    and b' = beta - A*mean folded into a dual-op tensor_scalar (also drains
    PSUM), relu via tensor_scalar max 0.

kernel() is self-contained: builds the Bass program once, shards inputs on host,
runs via bass_utils.run_bass_kernel_spmd on cores 0..7, reassembles full output.
"""

import functools
import os

import numpy as np

B, C, O, H, W = 4, 64, 64, 32, 32
K, PAD = 3, 1
HP, WP = H + 2 * PAD, W + 2 * PAD  # 34, 34
L = H * W  # 1024
SPP = HP * WP  # 1156 padded spatial per batch
NCORES = 8
O_PER = O // NCORES  # 8
NB2 = B // 2  # bpairs
EPS = 1e-5
NSTAT = O_PER * B  # 32 rows of S
NPIX = B * L  # 4096 values per channel for BN stats

# engine split knobs: 72 taps per core (8 o x 9 taps).
N_ACT_OPS = int(os.environ.get("KRN_ACT_OPS", "29"))  # taps on ACT (1 op, ~1.9us)
N_MERGE = int(os.environ.get("KRN_MERGE", "5"))  # DVE tap-pairs pre-summed
DUAL_OP = int(os.environ.get("KRN_DUAL", "0"))  # sub+and in one tensor_scalar
D_BUFS = int(os.environ.get("KRN_D_BUFS", "10"))


def _engine_schedule(n_ops: int, n_act: int):
    """Return list of 'v'/'a' of length n_ops, interleaving engines evenly."""
    n_a = min(n_act, n_ops)
    n_v = n_ops - n_a
    counts = {"v": n_v, "a": n_a}
    acc = {k: 0.0 for k in counts}
    sched = []
    for _ in range(n_ops):
        for k in counts:
            acc[k] += counts[k] / n_ops
        pick = max(acc, key=lambda k: acc[k])
        acc[pick] -= 1.0
        sched.append(pick)
    return sched


def _merge_plan(sched):
    """Distribute N_MERGE pre-sum pairs round-robin over o's with >=2 DVE taps."""
    pairs = {o: [] for o in range(O_PER)}
    remaining = N_MERGE
    rnd = 0
    while remaining > 0 and rnd < 4:
        for o in range(O_PER):
            if remaining <= 0:
                break
            dve = [t for t in range(9) if sched[o * 9 + t] == "v"]
            used = {t for p in pairs[o] for t in p}
            free = [t for t in dve if t not in used]
            if len(free) >= 2:
                pairs[o].append((free[0], free[1]))
                remaining -= 1
        rnd += 1
    return pairs


def _emit_main(nc, tc, mybir, xph4, xpho4, wcols, nwcols, mselh, dpool, ps, sched, pairs):
    from concourse import mybir as _mb

    f16 = _mb.dt.float16
    u16 = _mb.dt.uint16

    def absdiff(o, tap):
        """Emit |x - w| for (o, tap) -> f16 tile d[128, NB2, H, W]."""
        ky, kx = tap // 3, tap % 3
        idx = o * 9 + tap
        eng = sched[idx]
        if kx == 1:
            # odd flat offset: read the +1-shifted image at kx-1 (even, aligned)
            view = xpho4[:, :, ky : ky + H, kx - 1 : kx - 1 + W]
        else:
            view = xph4[:, :, ky : ky + H, kx : kx + W]
        if eng == "a":
            # ACT reads either image; alignment irrelevant for ACT
            aview = xph4[:, :, ky : ky + H, kx : kx + W]
            d = dpool.tile([128, NB2, H, W], f16, tag="da", name=f"da{idx}")
            nc.scalar.activation(
                out=d[:], in_=aview, func=_mb.ActivationFunctionType.Abs,
                bias=nwcols[:, idx : idx + 1], scale=1.0,
            )
            return d
        if DUAL_OP:
            d = dpool.tile([128, NB2, H, W], f16, tag="dv", name=f"dv{idx}")
            nc.vector.tensor_scalar(
                out=d[:], in0=view, scalar1=wcols[:, idx : idx + 1],
                scalar2=0x7FFF, op0=_mb.AluOpType.subtract,
                op1=_mb.AluOpType.bitwise_and,
            )
            return d
        d1 = dpool.tile([128, NB2, H, W], f16, tag="d1", name=f"d1_{idx}")
        nc.vector.tensor_scalar_sub(d1[:], view, wcols[:, idx : idx + 1])
        d = dpool.tile([128, NB2, H, W], f16, tag="dv", name=f"dv{idx}")
        nc.vector.tensor_scalar(
            out=d[:].bitcast(u16), in0=d1[:].bitcast(u16),
            scalar1=0x7FFF, scalar2=None, op0=_mb.AluOpType.bitwise_and,
        )
        return d

    first = [True, True]

    def mm(o, d, last):
        d2 = d.rearrange("p u a b -> p (u a b)")
        for u in range(NB2):
            for half in range(2):
                nc.tensor.matmul(
                    ps[half][:, :],
                    lhsT=mselh[:, (o * 2 + u) * NSTAT : (o * 2 + u + 1) * NSTAT],
                    rhs=d2[:, u * L + half * 512 : u * L + half * 512 + 512],
                    start=first[half],
                    stop=(last and u == NB2 - 1),
                )
                first[half] = False

    for o in range(O_PER):
        last_of_o = o == O_PER - 1
        merged = {t for p in pairs[o] for t in p}
        # plain taps first: keeps the in-order PE queue fed while the
        # DVE merge chains (abs, abs, add) complete in the background
        plain = [absdiff(o, tap) for tap in range(9) if tap not in merged]
        for i, d in enumerate(plain):
            mm(o, d, last_of_o and not pairs[o] and i == len(plain) - 1)
        for k, (t1, t2) in enumerate(pairs[o]):
            di = absdiff(o, t1)
            dj = absdiff(o, t2)
            dsum = dpool.tile([128, NB2, H, W], f16, tag="dsum",
                              name=f"dsum{o}_{t1}")
            nc.vector.tensor_add(dsum[:], di[:], dj[:])
            mm(o, dsum, last_of_o and k == len(pairs[o]) - 1)


@functools.lru_cache(maxsize=4)
def _build_program(bench_iters=0):
    from contextlib import ExitStack

    import concourse.tile as tile
    from concourse import bacc, mybir

    f32 = mybir.dt.float32
    f16 = mybir.dt.float16

    nc = bacc.Bacc("TRN2", target_bir_lowering=False, debug=False)

    # pre-padded f16 images: partition p=(h,c), free=(u, y, x); xpho shifted +1
    xph_t = nc.dram_tensor("xph", (128, NB2 * SPP), f16, kind="ExternalInput")
    xpho_t = nc.dram_tensor("xpho", (128, NB2 * SPP), f16, kind="ExternalInput")
    # wpack[:, :72] = wcols (w[o_g, p%64, tap]), [:, 72:144] = -wcols
    wpack_t = nc.dram_tensor("wpack", (128, 2 * O_PER * 9), f32, kind="ExternalInput")
    # mselh[p, (o*2+u)*32 + j] = 1.0 iff j == o*4 + 2u + p//64
    mselh_t = nc.dram_tensor("mselh", (128, O_PER * 2 * NSTAT), f16, kind="ExternalInput")
    # spack[:, :32] = osel, [:, 32] = -gamma col, [:, 33] = beta col
    spack_t = nc.dram_tensor("spack", (NSTAT, NSTAT + 2), f32, kind="ExternalInput")
    out_t = nc.dram_tensor("out", (NSTAT, L), f32, kind="ExternalOutput")

    sched = _engine_schedule(O_PER * 9, N_ACT_OPS)
    pairs = _merge_plan(sched)

    with tile.TileContext(nc) as tc, ExitStack() as ctx:
        consts = ctx.enter_context(tc.tile_pool(name="consts", bufs=1))
        dpool = ctx.enter_context(tc.tile_pool(name="dpool", bufs=D_BUFS))
        spool = ctx.enter_context(tc.tile_pool(name="spool", bufs=2))
        psum_main = ctx.enter_context(tc.tile_pool(name="psum_main", bufs=1, space="PSUM"))
        psum_stat = ctx.enter_context(tc.tile_pool(name="psum_stat", bufs=2, space="PSUM"))

        # ---- inputs to SBUF ----
        xph = consts.tile([128, NB2 * SPP], f16)
        xpho = consts.tile([128, NB2 * SPP], f16)
        wpack = consts.tile([128, 2 * O_PER * 9], f32)
        mselh = consts.tile([128, O_PER * 2 * NSTAT], f16)
        spack = consts.tile([NSTAT, NSTAT + 2], f32)
        nc.sync.dma_start(out=xph[:], in_=xph_t[:, :])
        nc.sync.dma_start(out=xpho[:], in_=xpho_t[:, :])
        nc.sync.dma_start(out=wpack[:], in_=wpack_t[:, :])
        nc.sync.dma_start(out=mselh[:], in_=mselh_t[:, :])
        nc.sync.dma_start(out=spack[:], in_=spack_t[:, :])
        wcols = wpack[:, 0 : O_PER * 9]
        nwcols = wpack[:, O_PER * 9 : 2 * O_PER * 9]
        osel = spack[:, 0:NSTAT]
        gcol = spack[:, NSTAT : NSTAT + 1]
        bcol = spack[:, NSTAT + 1 : NSTAT + 2]

        xph4 = xph.rearrange("p (u a b) -> p u a b", u=NB2, a=HP, b=WP)
        xpho4 = xpho.rearrange("p (u a b) -> p u a b", u=NB2, a=HP, b=WP)

        # dummy ACT ops on a tiny tile: preload the epilogue's activation
        # table set during the input DMAs so the tail pays no table switch
        dumin = spool.tile([NSTAT, 1], f32, tag="dumin")
        nc.vector.memset(dumin[:], 1.0)
        dumout = spool.tile([NSTAT, 1], f32, tag="dumout")
        nc.scalar.activation(
            out=dumout[:], in_=dumin[:],
            func=mybir.ActivationFunctionType.Square,
        )
        nc.scalar.activation(
            out=dumout[:], in_=dumin[:],
            func=mybir.ActivationFunctionType.Abs_reciprocal_sqrt,
        )
        nc.scalar.activation(
            out=dumout[:], in_=dumin[:],
            func=mybir.ActivationFunctionType.Relu,
        )

        # ---- main loop: S[o*4+b, l] accumulates over taps in two PSUM halves
        ps_big = psum_main.tile([NSTAT, 2 * 512], f32, name="ps_big")
        ps = [ps_big[:, h * 512 : h * 512 + 512] for h in range(2)]
        import contextlib

        loop_cm = (
            tc.For_i(0, bench_iters, 1) if bench_iters else contextlib.nullcontext()
        )
        with loop_cm:
            _emit_main(nc, tc, mybir, xph4, xpho4, wcols, nwcols, mselh,
                       dpool, ps, sched, pairs)

        # ---- epilogue: BN stats + normalize + relu, all on DVE ----
        sums2 = spool.tile([NSTAT, 2], f32, tag="sums2")
        # drain S from PSUM to SBUF; accum_out gives the row sums for free
        scp = spool.tile([NSTAT, L], f32, tag="scp")
        nc.vector.tensor_scalar(
            out=scp[:], in0=ps_big[:], scalar1=1.0, scalar2=None,
            op0=mybir.AluOpType.mult, op1=mybir.AluOpType.add,
            accum_out=sums2[:, 0:1],
        )
        scr = spool.tile([NSTAT, L], f32, tag="scr")
        nc.vector.scalar_tensor_tensor(
            out=scr[:], in0=scp[:], scalar=1.0, in1=scp[:],
            op0=mybir.AluOpType.mult, op1=mybir.AluOpType.mult,
            accum_out=sums2[:, 1:2],
        )
        stat_ps = psum_stat.tile([NSTAT, 2], f32, tag="statps")
        nc.tensor.matmul(stat_ps[:], lhsT=osel, rhs=sums2[:], start=True, stop=True)
        mom = spool.tile([NSTAT, 2], f32, tag="mom")
        nc.vector.tensor_scalar_mul(mom[:], stat_ps[:], 1.0 / NPIX)  # [mean, E[S^2]]
        mean = mom[:, 0:1]
        nvar = spool.tile([NSTAT, 1], f32, tag="nvar")
        # (mean*mean) - E[S^2] = -var
        nc.vector.scalar_tensor_tensor(
            out=nvar[:], in0=mean, scalar=mean, in1=mom[:, 1:2],
            op0=mybir.AluOpType.mult, op1=mybir.AluOpType.subtract,
        )
        epsc = spool.tile([NSTAT, 1], f32, tag="epsc")
        nc.vector.memset(epsc[:], EPS)
        rstd = spool.tile([NSTAT, 1], f32, tag="rstd")
        # rstd = 1/sqrt(|-nvar + eps|) = 1/sqrt(var + eps)
        nc.scalar.activation(
            out=rstd[:], in_=nvar[:],
            func=mybir.ActivationFunctionType.Abs_reciprocal_sqrt,
            bias=epsc[:], scale=-1.0,
        )
        acol = spool.tile([NSTAT, 1], f32, tag="acol")
        nc.vector.tensor_mul(acol[:], gcol, rstd[:])  # A = -gamma*rstd
        qcol = spool.tile([NSTAT, 1], f32, tag="qcol")
        # q = A*mean - beta ; out = relu(A*S - q)
        nc.vector.scalar_tensor_tensor(
            out=qcol[:], in0=acol[:], scalar=mean, in1=bcol,
            op0=mybir.AluOpType.mult, op1=mybir.AluOpType.subtract,
        )
        t1 = spool.tile([NSTAT, L], f32, tag="t1")
        nc.vector.tensor_scalar(
            out=t1[:], in0=scp[:], scalar1=acol[:], scalar2=qcol[:],
            op0=mybir.AluOpType.mult, op1=mybir.AluOpType.subtract,
        )
        outf = spool.tile([NSTAT, L], f32, tag="outf")
        nc.vector.tensor_scalar_max(outf[:], t1[:], 0.0)
        nc.sync.dma_start(out=out_t[:, :], in_=outf[:])

    nc.compile()
    return nc


def _host_inputs(x, weight, gamma, beta):
    """Build the 8 per-core input maps."""
    x = np.ascontiguousarray(x, dtype=np.float32)
    weight = np.asarray(weight, dtype=np.float32)
    gamma = np.asarray(gamma, dtype=np.float32)
    beta = np.asarray(beta, dtype=np.float32)

    # padded f16 images, shared by all cores
    img = np.zeros((2, C, NB2, HP, WP), dtype=np.float16)
    for b in range(B):
        h, u = b % 2, b // 2
        img[h, :, u, PAD : PAD + H, PAD : PAD + W] = x[b].astype(np.float16)
    xph = img.reshape(128, NB2 * SPP)
    xpho = np.zeros_like(xph)
    xpho[:, : NB2 * SPP - 1] = xph[:, 1:]

    msel = np.zeros((128, O_PER * 2 * NSTAT), dtype=np.float32)
    for o in range(O_PER):
        for u in range(NB2):
            for p_half in range(2):
                j = o * 4 + 2 * u + p_half
                col = (o * 2 + u) * NSTAT + j
                msel[p_half * 64 : (p_half + 1) * 64, col] = 1.0
    osel = np.zeros((NSTAT, NSTAT), dtype=np.float32)
    for p in range(NSTAT):
        for m in range(NSTAT):
            if p // B == m // B:
                osel[p, m] = 1.0

    in_maps = []
    for core in range(NCORES):
        osl = slice(core * O_PER, (core + 1) * O_PER)
        w = weight[osl]  # [8, 64, 3, 3]
        # wcols[p, o*9+tap] = w[o, p%64, tap//3, tap%3]
        wc = w.reshape(O_PER, C, 9).transpose(1, 0, 2).reshape(C, O_PER * 9)
        wcols = np.concatenate([wc, wc], axis=0).astype(np.float32)  # [128, 72]
        wpack = np.concatenate([wcols, -wcols], axis=1)  # [128, 144]
        gcol = np.repeat(-gamma[osl], B).reshape(NSTAT, 1).astype(np.float32)
        bcol = np.repeat(beta[osl], B).reshape(NSTAT, 1).astype(np.float32)
        spack = np.concatenate([osel, gcol, bcol], axis=1)  # [32, 34]
        in_maps.append(
            {
                "xph": xph,
                "xpho": xpho,
                "wpack": np.ascontiguousarray(wpack),
                "mselh": msel.astype(np.float16),
                "spack": np.ascontiguousarray(spack),
            }
        )
    return in_maps


def _assemble(results):
    out = np.empty((B, O, H, W), dtype=np.float32)
    for core, res in enumerate(results):
        arr = res["out"].reshape(O_PER, B, H, W)  # row = o*4+b
        out[:, core * O_PER : (core + 1) * O_PER] = arr.transpose(1, 0, 2, 3)
    return out


def kernel(x, weight, gamma, beta, _trace=False):
    from concourse import bass_utils

    nc = _build_program()
    in_maps = _host_inputs(x, weight, gamma, beta)
    res = bass_utils.run_bass_kernel_spmd(
        nc, in_maps, core_ids=list(range(NCORES)), trace=_trace
    )
    out = _assemble(res.results)
    if _trace:
        return out, res
    return out
